# revision 62
# baseline (speedup 1.0000x reference)
"""GQA attention (B=1, S=2048, D=2048, H=32, KV=4, HD=64) on 8 TRN2 NeuronCores.

Sharding: tensor-parallel over heads. Core c owns q-heads [4c, 4c+4) and kv-head
c//2 (all four q-heads of a core share one kv head). Single fused pipeline per
512-query chunk c:
  1. x chunk cast-DMA (f32->bf16, gpsimd SWDGE, prefetched one chunk ahead);
     x^T via PE-transpose (identity matmul; DMA-xbar transposes are force-
     serialized against collectives by the tile scheduler, so they can't be
     used here), PSUM->SBUF copies split 12/4 across ScalarE/VectorE.
  2. Projections Q^T / [K^T; V^T] (weights stationary, contract D on PE).
  3. RoPE applied by DVE directly on the PSUM projection result (head-dim
     pre-permuted to [evens|odds] in the weight columns): one full-tile cos
     mul + 32-row-shifted sin muls + combines; K duplicated to rows 64:128
     for the PE row-group pair; V^T -> V via PE transpose into stride-80
     blocks with a ones column.
  4. Flash-style causal attention for q-chunk c over key blocks 0..4c+4:
     scores^T via PE (two heads packed per PSUM tile in col halves, PE row
     groups 0/64 run concurrently), exp on ScalarE (3D AP covering both
     heads, causally N-trimmed per diagonal block), in-block triangle mask
     via gpsimd affine_select, PV lagging scores by two blocks (hides exp
     latency), with a ones-column on V so softmax denominators fall out of
     the same matmul. Normalization AT = pv * PE-broadcast(1/den) is read
     straight from PSUM; mb0's broadcast is interleaved into mb1's loop.
  5. Chunked AllGather of attn^T per query chunk (gpsimd-triggered, inputs
     staged via ScalarE DMA); output projection for chunk k trails two
     chunks later; the last chunk's gather-tile loads interleave with its
     projection quads.
Returns out^T [256, 2048] per core; the host transposes/concatenates.
"""

import numpy as np

import concourse.bass as bass
import concourse.mybir as mybir
import concourse.tile as tile
from concourse import bacc
from concourse import bass_utils
from concourse.masks import make_identity

F32 = mybir.dt.float32
BF16 = mybir.dt.bfloat16
AF = mybir.ActivationFunctionType
ALU = mybir.AluOpType

S = 2048
D = 2048
HD = 64
CORES = 8
SC = 512
NSC = S // SC
NDC = D // 128

_NC_CACHE = {}


def _dram3(t, row0, nrow_p, nblk, blk_stride, ncol, col0=0):
    """AP over DRAM tensor t: [partition p, block b, col f] ->
    t[row0 + b*blk_stride + p, col0 + f], p<nrow_p, b<nblk, f<ncol."""
    if isinstance(t, bass.AP):
        handle, off0, row_pitch = t.tensor, t.offset, t.tensor.shape[1]
    else:
        handle, off0, row_pitch = t, 0, t.shape[1]
    return bass.AP(
        handle,
        off0 + row0 * row_pitch + col0,
        [[row_pitch, nrow_p], [blk_stride * row_pitch, nblk], [1, ncol]],
    )


def build():
    if "nc" in _NC_CACHE:
        return _NC_CACHE["nc"]
    nc = bacc.Bacc(None, target_bir_lowering=False, debug=False)

    x = nc.declare_dram_parameter("x", [S, D], F32, isOutput=False)
    wq = nc.declare_dram_parameter("wq", [D, 256], F32, isOutput=False)
    wkv = nc.declare_dram_parameter("wkv", [D, 128], F32, isOutput=False)
    wo = nc.declare_dram_parameter("wo", [D, 256], F32, isOutput=False)
    cs = nc.declare_dram_parameter("cs", [32, 2 * S], F32, isOutput=False)
    out = nc.declare_dram_parameter("out", [256, S], F32, isOutput=True)

    with tile.TileContext(nc) as tc:
        with (
            tc.tile_pool(name="const", bufs=1) as const,
            tc.tile_pool(name="wpool", bufs=1) as wpool,
            tc.tile_pool(name="big", bufs=1) as big,
            tc.tile_pool(name="xp", bufs=2) as xp,
            tc.tile_pool(name="xtp", bufs=1) as xtp,
            tc.tile_pool(name="rp", bufs=4) as rp,
            tc.tile_pool(name="vbp", bufs=2) as vbp,
            tc.tile_pool(name="ptp", bufs=4) as ptp,
            tc.tile_pool(name="npool", bufs=6) as npool,
            tc.tile_pool(name="agt", bufs=8) as agtp,
            tc.tile_pool(name="otp", bufs=2) as otp,
            tc.tile_pool(name="projp", bufs=2, space="PSUM") as projp,
            tc.tile_pool(name="scp", bufs=2, space="PSUM") as scp,
            tc.tile_pool(name="pvp", bufs=1, space="PSUM") as pvp,
            tc.tile_pool(name="dram", bufs=1, space="DRAM") as dram,
        ):
            # ---- x chunk 0 first (critical path), then weights ----
            idb = const.tile([128, 128], BF16)
            make_identity(nc, idb[:])
            xbf0 = xp.tile([128, 4 * D], BF16, name="xbf", tag="xbf")
            for rb in range(4):
                nc.gpsimd.dma_start(
                    out=xbf0[:, rb * D : rb * D + D],
                    in_=x[rb * 128 : rb * 128 + 128, :],
                )
            wqb = wpool.tile([128, NDC * 256], BF16)
            wkvb = wpool.tile([128, NDC * 128], BF16)
            nc.gpsimd.dma_start(out=wkvb[:], in_=_dram3(wkv, 0, 128, NDC, 128, 128))
            nc.gpsimd.dma_start(out=wqb[:], in_=_dram3(wq, 0, 128, NDC, 128, 256))
            wob = wpool.tile([128, NDC * 256], BF16)

            # ---- constants ----
            ones64 = const.tile([1, 64], BF16)
            nc.vector.memset(ones64[:], 1.0)
            # cos/sin on 4 partition bands: rows 32b..32b+32: cols [0,S)=cos^T,
            # [S,2S)=sin^T
            cs4 = const.tile([128, 2 * S], BF16)
            for b in range(4):
                nc.gpsimd.dma_start(out=cs4[32 * b : 32 * b + 32, :], in_=cs[:, :])

            xbf1 = xp.tile([128, 4 * D], BF16, name="xbf", tag="xbf")
            for h in range(2):
                nc.gpsimd.dma_start(
                    out=bass.AP(
                        xbf1.tensor,
                        xbf1.offset + h * 2 * D,
                        [xbf1.ap[0], [D, 2], [1, D]],
                    ),
                    in_=_dram3(x, SC + h * 256, 128, 2, 128, D),
                )

            # ---- persistent activations ----
            QT = [big.tile([128, S], BF16, name=f"QT{i}") for i in range(2)]
            KT2 = big.tile([128, S], BF16)
            # V blocks at stride 80 (xbar out col offsets must be 32B-aligned;
            # 65*2B is not): cols [80j, 80j+64) = V block j, col 80j+64 = ones
            Vext = big.tile([128, (S // 128) * 80], BF16)
            nc.vector.memset(Vext[:], 1.0)
            AT = [big.tile([128, S], BF16, name=f"AT{i}") for i in range(2)]

            ag_in = [dram.tile([256, SC], BF16, name=f"ag_in{i}") for i in range(NSC)]
            ag_out = [
                dram.tile([CORES * 256, SC], BF16, addr_space="Shared", name=f"ag_out{i}")
                for i in range(NSC)
            ]

            def rope128(psrc, r0, nr, dst, dcol, ccol, dup64=False):
                """RoPE on psrc[r0:r0+nr] (PSUM f32, rows = head-pairs of
                [E(32)|O(32)]); writes dst[r0:r0+nr, dcol:dcol+SC] bf16.
                TT inputs must share a start partition (BIR verifier), so the
                cross-32-row sin products shift via the *output* base."""
                t1 = rp.tile([128, SC], BF16, name="t1", tag="t1")
                t2 = rp.tile([128, SC], BF16, name="t2", tag="t2")
                nc.vector.tensor_mul(
                    t1[r0 : r0 + nr, :], psrc[r0 : r0 + nr, :],
                    cs4[r0 : r0 + nr, ccol : ccol + SC],
                )
                for h0 in range(r0, r0 + nr, 64):
                    # t2[E rows] = O*sin ; t2[O rows] = E*sin
                    nc.vector.tensor_mul(
                        t2[h0 : h0 + 32, :], psrc[h0 + 32 : h0 + 64, :],
                        cs4[h0 + 32 : h0 + 64, S + ccol : S + ccol + SC],
                    )
                    nc.vector.tensor_mul(
                        t2[h0 + 32 : h0 + 64, :], psrc[h0 : h0 + 32, :],
                        cs4[h0 : h0 + 32, S + ccol : S + ccol + SC],
                    )
                    # E' = E*cos - O*sin ; O' = O*cos + E*sin
                    nc.vector.tensor_sub(
                        dst[h0 : h0 + 32, dcol : dcol + SC],
                        t1[h0 : h0 + 32, :],
                        t2[h0 : h0 + 32, :],
                    )
                    nc.vector.tensor_add(
                        dst[h0 + 32 : h0 + 64, dcol : dcol + SC],
                        t1[h0 + 32 : h0 + 64, :],
                        t2[h0 + 32 : h0 + 64, :],
                    )
                    if dup64:
                        # duplicate into rows +64 (for the PE row-group pair)
                        nc.gpsimd.tensor_sub(
                            dst[h0 + 64 : h0 + 96, dcol : dcol + SC],
                            t1[h0 : h0 + 32, :],
                            t2[h0 : h0 + 32, :],
                        )
                        nc.gpsimd.tensor_add(
                            dst[h0 + 96 : h0 + 128, dcol : dcol + SC],
                            t1[h0 + 32 : h0 + 64, :],
                            t2[h0 + 32 : h0 + 64, :],
                        )

            atsd = {}

            def ats_loads(k):
                ats = []
                for e4 in range(4):
                    t = agtp.tile([128, 4 * SC], BF16, name="ats", tag="ats")
                    nc.sync.dma_start(
                        out=t[:], in_=_dram3(ag_out[k], e4 * 512, 128, 4, 128, SC)
                    )
                    ats.append(t)
                atsd[k] = ats

            def outproj(k):
                ats = atsd[k]
                for mb in range(2):
                    po = projp.tile([128, SC], F32, name="po", tag="proj")
                    for e in range(16):
                        nc.tensor.matmul(
                            po[:],
                            wob[:, e * 256 + mb * 128 : e * 256 + mb * 128 + 128],
                            ats[e // 4][:, (e % 4) * SC : (e % 4) * SC + SC],
                            start=(e == 0),
                            stop=(e == 15),
                        )
                    oT = otp.tile([128, SC], F32, name="oT", tag="oT")
                    nc.vector.tensor_copy(oT[:], po[:])
                    nc.sync.dma_start(
                        out=out[mb * 128 : mb * 128 + 128, k * SC : k * SC + SC],
                        in_=oT[:],
                    )

            xbfs = {0: xbf0, 1: xbf1}

            def ltp(c):
                """x transposes + QKV projections + rope for chunk c."""
                if c + 1 < NSC and (c + 1) not in xbfs:
                    xn = xp.tile([128, 4 * D], BF16, name="xbf", tag="xbf")
                    for h in range(2):
                        nc.gpsimd.dma_start(
                            out=bass.AP(
                                xn.tensor,
                                xn.offset + h * 2 * D,
                                [xn.ap[0], [D, 2], [1, D]],
                            ),
                            in_=_dram3(x, (c + 1) * SC + h * 256, 128, 2, 128, D),
                        )
                    xbfs[c + 1] = xn
                xbf = xbfs.pop(c)
                # xT[p, dc*SC + rb*128 + s] = x[c*SC + rb*128 + s, dc*128 + p]
                xT = xtp.tile([128, NDC * SC], BF16, name="xT", tag="xT")
                for dc in range(NDC):
                    tp = projp.tile([128, SC], F32, name="tp", tag="proj")
                    for rb in range(4):
                        nc.tensor.matmul(
                            tp[:, rb * 128 : rb * 128 + 128],
                            xbf[:, rb * D + dc * 128 : rb * D + dc * 128 + 128],
                            idb[:],
                            start=True,
                            stop=True,
                        )
                    if dc % 4 != 3:
                        nc.scalar.activation(
                            xT[:, dc * SC : dc * SC + SC], tp[:], AF.Copy
                        )
                    else:
                        nc.vector.tensor_copy(xT[:, dc * SC : dc * SC + SC], tp[:])

                # ---- projections: KV first, then Q(mb0), Q(mb1) ----
                pskv = projp.tile([128, SC], F32, name="pskv", tag="proj")
                for dc in range(NDC):
                    nc.tensor.matmul(
                        pskv[:],
                        wkvb[:, dc * 128 : dc * 128 + 128],
                        xT[:, dc * SC : dc * SC + SC],
                        start=(dc == 0),
                        stop=(dc == NDC - 1),
                    )
                # K rope -> KT2[0:64] + dup to [64:128]; V -> vbf -> PE -> Vext
                rope128(pskv, 0, 64, KT2, c * SC, c * SC, dup64=True)
                vbf = vbp.tile([64, SC], BF16, name="vbf", tag="vbf")
                nc.vector.tensor_copy(vbf[:], pskv[64:128, :])
                tv = projp.tile([128, SC], F32, name="tv", tag="proj")
                for jj in range(4):
                    nc.tensor.matmul(
                        tv[:, jj * 64 : jj * 64 + 64],
                        vbf[:, jj * 128 : jj * 128 + 128],
                        idb[0:64, 0:64],
                        start=True,
                        stop=True,
                    )
                vdst = bass.AP(
                    Vext.tensor,
                    Vext.offset + (c * 4) * 80,
                    [Vext.ap[0], [80, 4], [1, 64]],
                )
                vsrc = bass.AP(tv.tensor, tv.offset, [tv.ap[0], [64, 4], [1, 64]])
                nc.vector.tensor_copy(vdst, vsrc)

                for mb in range(2):
                    p = projp.tile([128, SC], F32, name="psq", tag="proj")
                    for dc in range(NDC):
                        nc.tensor.matmul(
                            p[:],
                            wqb[:, dc * 256 + mb * 128 : dc * 256 + mb * 128 + 128],
                            xT[:, dc * SC : dc * SC + SC],
                            start=(dc == 0),
                            stop=(dc == NDC - 1),
                        )
                    rope128(p, 0, 128, QT[mb], c * SC, c * SC)

            def att(c):
                """flash attention for q-chunk c + its AllGather."""

                def denom_pre(pv_t):
                    # vector: den -> 1/den (bf16) per lh
                    rbs = []
                    for lh in range(2):
                        den = npool.tile([1, SC], F32, name="den", tag="den")
                        nc.vector.tensor_copy(
                            den[:], pv_t[64:65, lh * SC : lh * SC + SC]
                        )
                        rf = npool.tile([1, SC], F32, name="rf", tag="rf")
                        nc.vector.reciprocal_approx_fast(rf[:], den[:])
                        rb_ = npool.tile([1, SC], BF16, name="rb", tag="rb")
                        nc.vector.tensor_copy(rb_[:], rf[:])
                        rbs.append(rb_)
                    return rbs

                def denom_post(pv_t, mb_, rbs):
                    # PE broadcast of 1/den + fused normalize into AT
                    for lh in range(2):
                        dbt = scp.tile([128, 2 * SC], F32, name="sct", tag="sct")
                        dbc = dbt[0:64, 0:SC]
                        nc.tensor.matmul(
                            dbc, ones64[:], rbs[lh][:], start=True, stop=True
                        )
                        dsb = npool.tile([64, SC], BF16, name="dsb", tag="dsb")
                        nc.vector.tensor_copy(dsb[:], dbc)
                        nc.vector.tensor_mul(
                            AT[mb_][64 * lh : 64 * lh + 64, c * SC : c * SC + SC],
                            pv_t[0:64, lh * SC : lh * SC + SC],
                            dsb[:],
                        )

                pending_norm = None
                nblk = 4 * c + 4
                for mb in range(2):
                    pv = pvp.tile([65, 2 * SC], F32, name="pv", tag="pv")

                    def pv_mm(j, pt, q0):
                        # PSUM out can't cross a bank: one matmul per lh half
                        for lh in range(2):
                            nc.tensor.matmul(
                                pv[:, lh * SC + q0 : lh * SC + SC],
                                Vext[:, j * 80 : j * 80 + 65],
                                pt[:, lh * SC + q0 : lh * SC + SC],
                                start=(j == 0),
                                stop=(j == nblk - 1),
                            )

                    pends = []  # software pipeline: PV lags scores by two j
                    for j in range(nblk):
                        q0 = max(0, (j - 4 * c) * 128)
                        nq = SC - q0
                        sct = scp.tile([128, 2 * SC], F32, name="sct", tag="sct")
                        for lh in range(2):
                            r0 = 64 * lh
                            nc.tensor.matmul(
                                sct[:, lh * SC + q0 : lh * SC + SC],
                                KT2[r0 : r0 + 64, j * 128 : j * 128 + 128],
                                QT[mb][r0 : r0 + 64, c * SC + q0 : c * SC + SC],
                                start=True,
                                stop=True,
                            )
                        pt = ptp.tile([128, 2 * SC], BF16, name="pt", tag="pt")
                        sc3 = bass.AP(
                            sct.tensor, sct.offset + q0,
                            [sct.ap[0], [SC, 2], [1, nq]],
                        )
                        pt3 = bass.AP(
                            pt.tensor, pt.offset + q0,
                            [pt.ap[0], [SC, 2], [1, nq]],
                        )
                        nc.scalar.activation(pt3, sc3, AF.Exp, scale=0.125)
                        if q0 or j == 4 * c:
                            # in-block triangle: keep q >= k
                            ptm = bass.AP(
                                pt.tensor, pt.offset + q0,
                                [pt.ap[0], [SC, 2], [1, 128]],
                            )
                            nc.gpsimd.affine_select(
                                out=ptm,
                                in_=ptm,
                                compare_op=ALU.is_gt,
                                fill=0.0,
                                base=1,
                                pattern=[[0, 2], [1, 128]],
                                channel_multiplier=-1,
                            )
                        pends.append((j, pt, q0))
                        if len(pends) > 2:
                            pv_mm(*pends.pop(0))
                        if mb == 1 and j == 1 and pending_norm is not None:
                            # mb0's denominator broadcast lands here so the PE
                            # never stalls on the vector reciprocal chain
                            denom_post(*pending_norm)
                            pending_norm = None
                    for p_ in pends:
                        pv_mm(*p_)
                    rbs = denom_pre(pv)
                    if mb == 0:
                        pending_norm = (pv, 0, rbs)
                    else:
                        if pending_norm is not None:
                            denom_post(*pending_norm)
                            pending_norm = None
                        denom_post(pv, 1, rbs)

                # ---- chunked AllGather ----
                for mb in range(2):
                    nc.scalar.dma_start(
                        out=ag_in[c][mb * 128 : mb * 128 + 128, :],
                        in_=AT[mb][:, c * SC : c * SC + SC],
                    )
                nc.gpsimd.collective_compute(
                    "AllGather",
                    ALU.bypass,
                    ins=[ag_in[c].opt()],
                    outs=[ag_out[c].opt()],
                    replica_groups=[list(range(CORES))],
                )

            # chunk-sequential schedule; outproj(k) trails by two chunks
            for c in range(NSC):
                if c >= 2:
                    ats_loads(c - 2)
                ltp(c)
                att(c)
                if c == 1:
                    nc.gpsimd.dma_start(
                        out=wob[:], in_=_dram3(wo, 0, 128, NDC, 128, 256)
                    )
                if c >= 2:
                    outproj(c - 2)
            ats_loads(2)
            outproj(2)
            # chunk 3: interleave gather-tile loads with outproj quads so the
            # first matmuls start as soon as the first tile lands
            ats3 = []
            po3 = [projp.tile([128, SC], F32, name="po", tag="proj") for _ in range(2)]
            for e4 in range(4):
                t = agtp.tile([128, 4 * SC], BF16, name="ats", tag="ats")
                nc.sync.dma_start(
                    out=t[:], in_=_dram3(ag_out[3], e4 * 512, 128, 4, 128, SC)
                )
                ats3.append(t)
                for mb in range(2):
                    for e in range(4 * e4, 4 * e4 + 4):
                        nc.tensor.matmul(
                            po3[mb][:],
                            wob[:, e * 256 + mb * 128 : e * 256 + mb * 128 + 128],
                            ats3[e4][:, (e % 4) * SC : (e % 4) * SC + SC],
                            start=(e == 0),
                            stop=(e == 15),
                        )
            for mb in range(2):
                oT = otp.tile([128, SC], F32, name="oT", tag="oT")
                nc.vector.tensor_copy(oT[:], po3[mb][:])
                nc.sync.dma_start(
                    out=out[mb * 128 : mb * 128 + 128, 3 * SC : 3 * SC + SC],
                    in_=oT[:],
                )

    nc.compile()
    _NC_CACHE["nc"] = nc
    return nc


_PERM = np.concatenate([np.arange(0, HD, 2), np.arange(1, HD, 2)])


def _shard_inputs(x, freqs_cos, freqs_sin, mask, wq, wk, wv, wo):
    x2 = np.ascontiguousarray(x.reshape(S, D), dtype=np.float32)
    cs = np.ascontiguousarray(
        np.concatenate([freqs_cos.T, freqs_sin.T], axis=1), dtype=np.float32
    )
    in_maps = []
    for c in range(CORES):
        g = c // 2
        wq_c = wq[:, 256 * c : 256 * c + 256].reshape(D, 4, HD)[:, :, _PERM]
        wq_c = np.ascontiguousarray(wq_c.reshape(D, 256), dtype=np.float32)
        wk_g = wk[:, HD * g : HD * g + HD][:, _PERM]
        wkv_c = np.ascontiguousarray(
            np.concatenate([wk_g, wv[:, HD * g : HD * g + HD]], axis=1),
            dtype=np.float32,
        )
        wo_c = np.ascontiguousarray(wo[:, 256 * c : 256 * c + 256], dtype=np.float32)
        in_maps.append({"x": x2, "wq": wq_c, "wkv": wkv_c, "wo": wo_c, "cs": cs})
    return in_maps


def kernel(x, freqs_cos, freqs_sin, mask, wq, wk, wv, wo, _trace=False):
    nc = build()
    in_maps = _shard_inputs(x, freqs_cos, freqs_sin, mask, wq, wk, wv, wo)
    res = bass_utils.run_bass_kernel_spmd(
        nc, in_maps, core_ids=list(range(CORES)), trace=_trace
    )
    outp = np.empty((S, D), dtype=np.float32)
    for c in range(CORES):
        outp[:, 256 * c : 256 * c + 256] = res.results[c]["out"].T
    if _trace:
        kernel._last_exec_time_ns = res.exec_time_ns
        kernel._last_results = res
    return outp.reshape(1, S, D)


# revision 63
# speedup vs baseline: 1.1200x; 1.1200x over previous
"""GQA attention (B=1, S=2048, D=2048, H=32, KV=4, HD=64) on 8 TRN2 NeuronCores.

Sharding: tensor-parallel over heads. Core c owns q-heads [4c, 4c+4) and kv-head
c//2 (all four q-heads of a core share one kv head). Single fused pipeline per
512-query chunk c:
  1. x chunk cast-DMA (f32->bf16, gpsimd SWDGE, prefetched one chunk ahead);
     x^T via PE-transpose (identity matmul; DMA-xbar transposes are force-
     serialized against collectives by the tile scheduler, so they can't be
     used here), PSUM->SBUF copies split 12/4 across ScalarE/VectorE.
  2. Projections Q^T / [K^T; V^T] (weights stationary, contract D on PE).
  3. RoPE applied by DVE directly on the PSUM projection result (head-dim
     pre-permuted to [evens|odds] in the weight columns): one full-tile cos
     mul + 32-row-shifted sin muls + combines; K duplicated to rows 64:128
     for the PE row-group pair; V^T -> V via PE transpose into stride-80
     blocks with a ones column.
  4. Flash-style causal attention for q-chunk c over key blocks 0..4c+4:
     scores^T via PE (two heads packed per PSUM tile in col halves, PE row
     groups 0/64 run concurrently), exp on ScalarE (3D AP covering both
     heads, causally N-trimmed per diagonal block), in-block triangle mask
     via gpsimd affine_select, PV lagging scores by two blocks (hides exp
     latency), with a ones-column on V so softmax denominators fall out of
     the same matmul. Normalization AT = pv * PE-broadcast(1/den) is read
     straight from PSUM; mb0's broadcast is interleaved into mb1's loop.
  5. Chunked AllGather of attn^T per query chunk (gpsimd-triggered, inputs
     staged via ScalarE DMA); output projection for chunk k trails two
     chunks later; the last chunk's gather-tile loads interleave with its
     projection quads.
Returns out^T [256, 2048] per core; the host transposes/concatenates.
"""

import numpy as np

import concourse.bass as bass
import concourse.mybir as mybir
import concourse.tile as tile
from concourse import bacc
from concourse import bass_utils
from concourse.masks import make_identity

F32 = mybir.dt.float32
BF16 = mybir.dt.bfloat16
AF = mybir.ActivationFunctionType
ALU = mybir.AluOpType

S = 2048
D = 2048
HD = 64
CORES = 8
SC = 512
NSC = S // SC
NDC = D // 128

_NC_CACHE = {}


def _dram3(t, row0, nrow_p, nblk, blk_stride, ncol, col0=0):
    """AP over DRAM tensor t: [partition p, block b, col f] ->
    t[row0 + b*blk_stride + p, col0 + f], p<nrow_p, b<nblk, f<ncol."""
    if isinstance(t, bass.AP):
        handle, off0, row_pitch = t.tensor, t.offset, t.tensor.shape[1]
    else:
        handle, off0, row_pitch = t, 0, t.shape[1]
    return bass.AP(
        handle,
        off0 + row0 * row_pitch + col0,
        [[row_pitch, nrow_p], [blk_stride * row_pitch, nblk], [1, ncol]],
    )


def build():
    if "nc" in _NC_CACHE:
        return _NC_CACHE["nc"]
    nc = bacc.Bacc(None, target_bir_lowering=False, debug=False)

    x = nc.declare_dram_parameter("x", [S, D], F32, isOutput=False)
    wq = nc.declare_dram_parameter("wq", [D, 256], F32, isOutput=False)
    wkv = nc.declare_dram_parameter("wkv", [D, 128], F32, isOutput=False)
    wo = nc.declare_dram_parameter("wo", [D, 256], F32, isOutput=False)
    cs = nc.declare_dram_parameter("cs", [32, 2 * S], F32, isOutput=False)
    out = nc.declare_dram_parameter("out", [256, S], F32, isOutput=True)

    with tile.TileContext(nc) as tc:
        with (
            tc.tile_pool(name="const", bufs=1) as const,
            tc.tile_pool(name="wpool", bufs=1) as wpool,
            tc.tile_pool(name="big", bufs=1) as big,
            tc.tile_pool(name="xp", bufs=2) as xp,
            tc.tile_pool(name="xtp", bufs=1) as xtp,
            tc.tile_pool(name="rp", bufs=4) as rp,
            tc.tile_pool(name="vbp", bufs=2) as vbp,
            tc.tile_pool(name="ptp", bufs=6) as ptp,
            tc.tile_pool(name="npool", bufs=6) as npool,
            tc.tile_pool(name="agt", bufs=8) as agtp,
            tc.tile_pool(name="otp", bufs=2) as otp,
            tc.tile_pool(name="projp", bufs=2, space="PSUM") as projp,
            tc.tile_pool(name="scp", bufs=2, space="PSUM") as scp,
            tc.tile_pool(name="pvp", bufs=1, space="PSUM") as pvp,
            tc.tile_pool(name="dram", bufs=1, space="DRAM") as dram,
        ):
            # ---- x chunk 0 first (critical path), then weights ----
            idb = const.tile([128, 128], BF16)
            make_identity(nc, idb[:])
            xbf0 = xp.tile([128, 4 * D], BF16, name="xbf", tag="xbf")
            for rb in range(4):
                nc.gpsimd.dma_start(
                    out=xbf0[:, rb * D : rb * D + D],
                    in_=x[rb * 128 : rb * 128 + 128, :],
                )
            wqb = wpool.tile([128, NDC * 256], BF16)
            wkvb = wpool.tile([128, NDC * 128], BF16)
            nc.gpsimd.dma_start(out=wkvb[:], in_=_dram3(wkv, 0, 128, NDC, 128, 128))
            nc.gpsimd.dma_start(out=wqb[:], in_=_dram3(wq, 0, 128, NDC, 128, 256))
            wob = wpool.tile([128, NDC * 256], BF16)

            # ---- constants ----
            ones64 = const.tile([1, 64], BF16)
            nc.vector.memset(ones64[:], 1.0)
            # cos/sin on 4 partition bands: rows 32b..32b+32: cols [0,S)=cos^T,
            # [S,2S)=sin^T
            cs4 = const.tile([128, 2 * S], BF16)
            for b in range(4):
                nc.gpsimd.dma_start(out=cs4[32 * b : 32 * b + 32, :], in_=cs[:, :])

            xbf1 = xp.tile([128, 4 * D], BF16, name="xbf", tag="xbf")
            for h in range(2):
                nc.gpsimd.dma_start(
                    out=bass.AP(
                        xbf1.tensor,
                        xbf1.offset + h * 2 * D,
                        [xbf1.ap[0], [D, 2], [1, D]],
                    ),
                    in_=_dram3(x, SC + h * 256, 128, 2, 128, D),
                )

            # ---- persistent activations ----
            QT = [big.tile([128, S], BF16, name=f"QT{i}") for i in range(2)]
            KT2 = big.tile([128, S], BF16)
            # V blocks at stride 80 (xbar out col offsets must be 32B-aligned;
            # 65*2B is not): cols [80j, 80j+64) = V block j, col 80j+64 = ones
            Vext = big.tile([128, (S // 128) * 80], BF16)
            nc.vector.memset(Vext[:], 1.0)
            AT = [big.tile([128, S], BF16, name=f"AT{i}") for i in range(2)]

            ag_in = [dram.tile([256, SC], BF16, name=f"ag_in{i}") for i in range(NSC)]
            ag_out = [
                dram.tile([CORES * 256, SC], BF16, addr_space="Shared", name=f"ag_out{i}")
                for i in range(NSC)
            ]

            def rope128(psrc, r0, nr, dst, dcol, ccol, dup64=False):
                """RoPE on psrc[r0:r0+nr] (PSUM f32, rows = head-pairs of
                [E(32)|O(32)]); writes dst[r0:r0+nr, dcol:dcol+SC] bf16.
                TT inputs must share a start partition (BIR verifier), so the
                cross-32-row sin products shift via the *output* base."""
                t1 = rp.tile([128, SC], BF16, name="t1", tag="t1")
                t2 = rp.tile([128, SC], BF16, name="t2", tag="t2")
                nc.vector.tensor_mul(
                    t1[r0 : r0 + nr, :], psrc[r0 : r0 + nr, :],
                    cs4[r0 : r0 + nr, ccol : ccol + SC],
                )
                for h0 in range(r0, r0 + nr, 64):
                    # t2[E rows] = O*sin ; t2[O rows] = E*sin
                    nc.vector.tensor_mul(
                        t2[h0 : h0 + 32, :], psrc[h0 + 32 : h0 + 64, :],
                        cs4[h0 + 32 : h0 + 64, S + ccol : S + ccol + SC],
                    )
                    nc.vector.tensor_mul(
                        t2[h0 + 32 : h0 + 64, :], psrc[h0 : h0 + 32, :],
                        cs4[h0 : h0 + 32, S + ccol : S + ccol + SC],
                    )
                    # E' = E*cos - O*sin ; O' = O*cos + E*sin
                    nc.vector.tensor_sub(
                        dst[h0 : h0 + 32, dcol : dcol + SC],
                        t1[h0 : h0 + 32, :],
                        t2[h0 : h0 + 32, :],
                    )
                    nc.vector.tensor_add(
                        dst[h0 + 32 : h0 + 64, dcol : dcol + SC],
                        t1[h0 + 32 : h0 + 64, :],
                        t2[h0 + 32 : h0 + 64, :],
                    )
                    if dup64:
                        # duplicate into rows +64 (for the PE row-group pair)
                        nc.gpsimd.tensor_sub(
                            dst[h0 + 64 : h0 + 96, dcol : dcol + SC],
                            t1[h0 : h0 + 32, :],
                            t2[h0 : h0 + 32, :],
                        )
                        nc.gpsimd.tensor_add(
                            dst[h0 + 96 : h0 + 128, dcol : dcol + SC],
                            t1[h0 + 32 : h0 + 64, :],
                            t2[h0 + 32 : h0 + 64, :],
                        )

            atsd = {}

            def ats_loads(k):
                ats = []
                for e4 in range(4):
                    t = agtp.tile([128, 4 * SC], BF16, name="ats", tag="ats")
                    nc.sync.dma_start(
                        out=t[:], in_=_dram3(ag_out[k], e4 * 512, 128, 4, 128, SC)
                    )
                    ats.append(t)
                atsd[k] = ats

            def outproj(k):
                ats = atsd[k]
                for mb in range(2):
                    po = projp.tile([128, SC], F32, name="po", tag="proj")
                    for e in range(16):
                        nc.tensor.matmul(
                            po[:],
                            wob[:, e * 256 + mb * 128 : e * 256 + mb * 128 + 128],
                            ats[e // 4][:, (e % 4) * SC : (e % 4) * SC + SC],
                            start=(e == 0),
                            stop=(e == 15),
                        )
                    oT = otp.tile([128, SC], F32, name="oT", tag="oT")
                    nc.vector.tensor_copy(oT[:], po[:])
                    nc.sync.dma_start(
                        out=out[mb * 128 : mb * 128 + 128, k * SC : k * SC + SC],
                        in_=oT[:],
                    )

            xbfs = {0: xbf0, 1: xbf1}

            def ltp(c):
                """x transposes + QKV projections + rope for chunk c."""
                if c + 1 < NSC and (c + 1) not in xbfs:
                    xn = xp.tile([128, 4 * D], BF16, name="xbf", tag="xbf")
                    for h in range(2):
                        nc.gpsimd.dma_start(
                            out=bass.AP(
                                xn.tensor,
                                xn.offset + h * 2 * D,
                                [xn.ap[0], [D, 2], [1, D]],
                            ),
                            in_=_dram3(x, (c + 1) * SC + h * 256, 128, 2, 128, D),
                        )
                    xbfs[c + 1] = xn
                xbf = xbfs.pop(c)
                # xT[p, dc*SC + rb*128 + s] = x[c*SC + rb*128 + s, dc*128 + p]
                xT = xtp.tile([128, NDC * SC], BF16, name="xT", tag="xT")
                for dc in range(NDC):
                    tp = projp.tile([128, SC], F32, name="tp", tag="proj")
                    for rb in range(4):
                        nc.tensor.matmul(
                            tp[:, rb * 128 : rb * 128 + 128],
                            xbf[:, rb * D + dc * 128 : rb * D + dc * 128 + 128],
                            idb[:],
                            start=True,
                            stop=True,
                        )
                    if dc % 4 != 3:
                        nc.scalar.activation(
                            xT[:, dc * SC : dc * SC + SC], tp[:], AF.Copy
                        )
                    else:
                        nc.vector.tensor_copy(xT[:, dc * SC : dc * SC + SC], tp[:])

                # ---- projections: KV first, then Q(mb0), Q(mb1) ----
                pskv = projp.tile([128, SC], F32, name="pskv", tag="proj")
                for dc in range(NDC):
                    nc.tensor.matmul(
                        pskv[:],
                        wkvb[:, dc * 128 : dc * 128 + 128],
                        xT[:, dc * SC : dc * SC + SC],
                        start=(dc == 0),
                        stop=(dc == NDC - 1),
                    )
                # K rope -> KT2[0:64] + dup to [64:128]; V -> vbf -> PE -> Vext
                rope128(pskv, 0, 64, KT2, c * SC, c * SC, dup64=True)
                vbf = vbp.tile([64, SC], BF16, name="vbf", tag="vbf")
                nc.vector.tensor_copy(vbf[:], pskv[64:128, :])
                tv = projp.tile([128, SC], F32, name="tv", tag="proj")
                for jj in range(4):
                    nc.tensor.matmul(
                        tv[:, jj * 64 : jj * 64 + 64],
                        vbf[:, jj * 128 : jj * 128 + 128],
                        idb[0:64, 0:64],
                        start=True,
                        stop=True,
                    )
                vdst = bass.AP(
                    Vext.tensor,
                    Vext.offset + (c * 4) * 80,
                    [Vext.ap[0], [80, 4], [1, 64]],
                )
                vsrc = bass.AP(tv.tensor, tv.offset, [tv.ap[0], [64, 4], [1, 64]])
                nc.vector.tensor_copy(vdst, vsrc)

                for mb in range(2):
                    p = projp.tile([128, SC], F32, name="psq", tag="proj")
                    for dc in range(NDC):
                        nc.tensor.matmul(
                            p[:],
                            wqb[:, dc * 256 + mb * 128 : dc * 256 + mb * 128 + 128],
                            xT[:, dc * SC : dc * SC + SC],
                            start=(dc == 0),
                            stop=(dc == NDC - 1),
                        )
                    rope128(p, 0, 128, QT[mb], c * SC, c * SC)

            def att(c):
                """flash attention for q-chunk c + its AllGather."""

                def denom_pre(pv_t):
                    # vector: den -> 1/den (bf16) per lh
                    rbs = []
                    for lh in range(2):
                        den = npool.tile([1, SC], F32, name="den", tag="den")
                        nc.vector.tensor_copy(
                            den[:], pv_t[64:65, lh * SC : lh * SC + SC]
                        )
                        rf = npool.tile([1, SC], F32, name="rf", tag="rf")
                        nc.vector.reciprocal_approx_fast(rf[:], den[:])
                        rb_ = npool.tile([1, SC], BF16, name="rb", tag="rb")
                        nc.vector.tensor_copy(rb_[:], rf[:])
                        rbs.append(rb_)
                    return rbs

                def denom_post(pv_t, mb_, rbs):
                    # PE broadcast of 1/den + fused normalize into AT
                    for lh in range(2):
                        dbt = scp.tile([128, 2 * SC], F32, name="sct", tag="sct")
                        dbc = dbt[0:64, 0:SC]
                        nc.tensor.matmul(
                            dbc, ones64[:], rbs[lh][:], start=True, stop=True
                        )
                        dsb = npool.tile([64, SC], BF16, name="dsb", tag="dsb")
                        nc.vector.tensor_copy(dsb[:], dbc)
                        nc.vector.tensor_mul(
                            AT[mb_][64 * lh : 64 * lh + 64, c * SC : c * SC + SC],
                            pv_t[0:64, lh * SC : lh * SC + SC],
                            dsb[:],
                        )

                pending_norm = None
                nblk = 4 * c + 4
                for mb in range(2):
                    pv = pvp.tile([65, 2 * SC], F32, name="pv", tag="pv")

                    def pv_mm(j, pt, q0):
                        # PSUM out can't cross a bank: one matmul per lh half
                        for lh in range(2):
                            nc.tensor.matmul(
                                pv[:, lh * SC + q0 : lh * SC + SC],
                                Vext[:, j * 80 : j * 80 + 65],
                                pt[:, lh * SC + q0 : lh * SC + SC],
                                start=(j == 0),
                                stop=(j == nblk - 1),
                            )

                    pends = []  # software pipeline: PV lags scores by two j
                    for j in range(nblk):
                        q0 = max(0, (j - 4 * c) * 128)
                        nq = SC - q0
                        sct = scp.tile([128, 2 * SC], F32, name="sct", tag="sct")
                        for lh in range(2):
                            r0 = 64 * lh
                            nc.tensor.matmul(
                                sct[:, lh * SC + q0 : lh * SC + SC],
                                KT2[r0 : r0 + 64, j * 128 : j * 128 + 128],
                                QT[mb][r0 : r0 + 64, c * SC + q0 : c * SC + SC],
                                start=True,
                                stop=True,
                            )
                        pt = ptp.tile([128, 2 * SC], BF16, name="pt", tag="pt")
                        sc3 = bass.AP(
                            sct.tensor, sct.offset + q0,
                            [sct.ap[0], [SC, 2], [1, nq]],
                        )
                        pt3 = bass.AP(
                            pt.tensor, pt.offset + q0,
                            [pt.ap[0], [SC, 2], [1, nq]],
                        )
                        nc.scalar.activation(pt3, sc3, AF.Exp, scale=0.125)
                        if q0 or j == 4 * c:
                            # in-block triangle: keep q >= k
                            ptm = bass.AP(
                                pt.tensor, pt.offset + q0,
                                [pt.ap[0], [SC, 2], [1, 128]],
                            )
                            nc.gpsimd.affine_select(
                                out=ptm,
                                in_=ptm,
                                compare_op=ALU.is_gt,
                                fill=0.0,
                                base=1,
                                pattern=[[0, 2], [1, 128]],
                                channel_multiplier=-1,
                            )
                        pends.append((j, pt, q0))
                        if len(pends) > 2:
                            pv_mm(*pends.pop(0))
                        if mb == 1 and j == 1 and pending_norm is not None:
                            # mb0's denominator broadcast lands here so the PE
                            # never stalls on the vector reciprocal chain
                            denom_post(*pending_norm)
                            pending_norm = None
                    for p_ in pends:
                        pv_mm(*p_)
                    rbs = denom_pre(pv)
                    if mb == 0:
                        pending_norm = (pv, 0, rbs)
                    else:
                        if pending_norm is not None:
                            denom_post(*pending_norm)
                            pending_norm = None
                        denom_post(pv, 1, rbs)

                # ---- chunked AllGather ----
                for mb in range(2):
                    nc.scalar.dma_start(
                        out=ag_in[c][mb * 128 : mb * 128 + 128, :],
                        in_=AT[mb][:, c * SC : c * SC + SC],
                    )
                nc.gpsimd.collective_compute(
                    "AllGather",
                    ALU.bypass,
                    ins=[ag_in[c].opt()],
                    outs=[ag_out[c].opt()],
                    replica_groups=[list(range(CORES))],
                )

            # chunk-sequential schedule; outproj(k) trails by two chunks
            for c in range(NSC):
                if c >= 2:
                    ats_loads(c - 2)
                ltp(c)
                att(c)
                if c == 1:
                    nc.gpsimd.dma_start(
                        out=wob[:], in_=_dram3(wo, 0, 128, NDC, 128, 256)
                    )
                if c >= 2:
                    outproj(c - 2)
            ats_loads(2)
            outproj(2)
            # chunk 3: interleave gather-tile loads with outproj quads so the
            # first matmuls start as soon as the first tile lands
            ats3 = []
            po3 = [projp.tile([128, SC], F32, name="po", tag="proj") for _ in range(2)]
            for e4 in range(4):
                t = agtp.tile([128, 4 * SC], BF16, name="ats", tag="ats")
                nc.sync.dma_start(
                    out=t[:], in_=_dram3(ag_out[3], e4 * 512, 128, 4, 128, SC)
                )
                ats3.append(t)
                for mb in range(2):
                    for e in range(4 * e4, 4 * e4 + 4):
                        nc.tensor.matmul(
                            po3[mb][:],
                            wob[:, e * 256 + mb * 128 : e * 256 + mb * 128 + 128],
                            ats3[e4][:, (e % 4) * SC : (e % 4) * SC + SC],
                            start=(e == 0),
                            stop=(e == 15),
                        )
            for mb in range(2):
                oT = otp.tile([128, SC], F32, name="oT", tag="oT")
                nc.vector.tensor_copy(oT[:], po3[mb][:])
                nc.sync.dma_start(
                    out=out[mb * 128 : mb * 128 + 128, 3 * SC : 3 * SC + SC],
                    in_=oT[:],
                )

    nc.compile()
    _NC_CACHE["nc"] = nc
    return nc


_PERM = np.concatenate([np.arange(0, HD, 2), np.arange(1, HD, 2)])


def _shard_inputs(x, freqs_cos, freqs_sin, mask, wq, wk, wv, wo):
    x2 = np.ascontiguousarray(x.reshape(S, D), dtype=np.float32)
    cs = np.ascontiguousarray(
        np.concatenate([freqs_cos.T, freqs_sin.T], axis=1), dtype=np.float32
    )
    in_maps = []
    for c in range(CORES):
        g = c // 2
        wq_c = wq[:, 256 * c : 256 * c + 256].reshape(D, 4, HD)[:, :, _PERM]
        wq_c = np.ascontiguousarray(wq_c.reshape(D, 256), dtype=np.float32)
        wk_g = wk[:, HD * g : HD * g + HD][:, _PERM]
        wkv_c = np.ascontiguousarray(
            np.concatenate([wk_g, wv[:, HD * g : HD * g + HD]], axis=1),
            dtype=np.float32,
        )
        wo_c = np.ascontiguousarray(wo[:, 256 * c : 256 * c + 256], dtype=np.float32)
        in_maps.append({"x": x2, "wq": wq_c, "wkv": wkv_c, "wo": wo_c, "cs": cs})
    return in_maps


def kernel(x, freqs_cos, freqs_sin, mask, wq, wk, wv, wo, _trace=False):
    nc = build()
    in_maps = _shard_inputs(x, freqs_cos, freqs_sin, mask, wq, wk, wv, wo)
    res = bass_utils.run_bass_kernel_spmd(
        nc, in_maps, core_ids=list(range(CORES)), trace=_trace
    )
    outp = np.empty((S, D), dtype=np.float32)
    for c in range(CORES):
        outp[:, 256 * c : 256 * c + 256] = res.results[c]["out"].T
    if _trace:
        kernel._last_exec_time_ns = res.exec_time_ns
        kernel._last_results = res
    return outp.reshape(1, S, D)
